# revision 13
# baseline (speedup 1.0000x reference)
"""Trainium2 8-core kernel for nn_Attention_88948772700322.

Reference computes (N=1024, B=4, C=1024, H=16, hd=64):
    qkv = x @ w_qkv.T                      [N,B,3C]
    q,k,v per (b,h); attn = softmax(q k^T / 8) v
    out = (attn.transpose(2,1,0,3)).reshape(N,B,C) @ w_proj.T + b_proj
The reshape interleaves H and B: proj-input channel c of output-batch bn is
attention head h = 4*bn + c//256, original batch b2 = (c%256)//64, dim d = c%64.

Sharding: tensor-parallel over heads — core i owns heads {2i, 2i+1}, all
batches/tokens (6.44 GFLOP/core, perfectly balanced).  Each core computes a
partial projection over its 512 proj-input channels for output batch bn=i//2;
host sums core pairs (the "all-reduce after proj" realized in unshard).

Host-side prep absorbs every layout nuisance:
  - xT [C, B*N] bf16, tokens batch-major  -> qkv needs no on-chip transpose
  - w_qk [C, 256] (cols q_h0,q_h1,k_h0,k_h1), q pre-scaled by 1/8
  - w_v  [C, 128] (cols v_h0,v_h1)
  - w_p  [512, 1024] = w_proj columns permuted to (b2, h_local, d) row order
On-chip per core: qk^T via PE (d-major), v via PE (token-major), scores
computed transposed (keys on partitions), softmax without max-subtraction
(scores are O(1) by construction), denominator via ones-column in V,
normalization by DMA-partition-broadcast reciprocal, partial proj n-major.
"""

import numpy as np
import ml_dtypes

import concourse.bass as bass
import concourse.mybir as mybir
from concourse import bacc
from concourse.tile import TileContext
from concourse.bass_utils import run_bass_kernel_spmd

N, B, C, H, HD = 1024, 4, 1024, 16, 64
NT = B * N          # 4096 tokens
NCORES = 8
BF = mybir.dt.bfloat16
F32 = mybir.dt.float32
bf16 = ml_dtypes.bfloat16

_NC_CACHE = {}
INST_PHASE = {}


def _T(phase, binst):
    INST_PHASE[binst.ins.name] = phase
    return binst


def _part_bcast(ap, nparts):
    """AP reading one partition, broadcast (step 0) across nparts partitions."""
    return bass.AP(tensor=ap.tensor, offset=ap.offset,
                   ap=[[0, nparts]] + list(ap.ap)[1:])


def build_nc():
    nc = bacc.Bacc()
    xT_e = nc.declare_dram_parameter("xT", [C, NT], BF, isOutput=False)
    wqk_e = nc.declare_dram_parameter("w_qk", [C, 256], BF, isOutput=False)
    wv_e = nc.declare_dram_parameter("w_v", [C, 128], BF, isOutput=False)
    wp_e = nc.declare_dram_parameter("w_p", [512, C], BF, isOutput=False)
    out_e = nc.declare_dram_parameter("out", [N, C], F32, isOutput=True)

    xT_ap = xT_e[:].rearrange("(co p) t -> p co t", p=128)    # [128, 8, 4096]
    wqk_ap = wqk_e[:].rearrange("(co p) m -> p co m", p=128)  # [128, 8, 256]
    wv_ap = wv_e[:].rearrange("(co p) m -> p co m", p=128)    # [128, 8, 128]
    wp_ap = wp_e[:].rearrange("(b2 p) d -> p b2 d", p=128)    # [128, 4, 1024]

    with TileContext(nc) as tc:
        with (
            tc.tile_pool(name="const", bufs=1) as cpool,
            tc.tile_pool(name="exp", bufs=4) as epool,
            tc.tile_pool(name="small", bufs=4) as spool,
            tc.tile_pool(name="outcp", bufs=3) as opool,
            tc.tile_pool(name="dram", bufs=2, space="DRAM") as dpool,
            tc.tile_pool(name="ps_qk", bufs=2, space="PSUM") as ps_qk,
            tc.tile_pool(name="ps_sT", bufs=2, space="PSUM") as ps_sT,
            tc.tile_pool(name="ps_av", bufs=2, space="PSUM") as ps_av,
        ):
            # ---- persistent SBUF tensors -------------------------------
            xT = cpool.tile([128, 8, NT], BF)      # x^T  [c_in, co, token]
            wqk = cpool.tile([128, 8, 256], BF)
            wv = cpool.tile([128, 8, 128], BF)
            wp = cpool.tile([128, 4, C], BF)
            q_sb = cpool.tile([128, NT], BF)       # [ (h0|h1) d, token ]
            k_sb = cpool.tile([128, NT], BF)
            # v token-major with ones cols: [t_in, t_out, (1,h0 d64,1,h1 d64)]
            v_sb = cpool.tile([128, 32, 130], BF)
            projin = cpool.tile([128, B, N], BF)   # [(hl,d), b2, n]

            nc.sync.dma_start(out=wqk[:], in_=wqk_ap)
            nc.sync.dma_start(out=wv[:], in_=wv_ap)
            # x chunks in batch-major order so batch 0 compute starts early
            for b in range(B):
                for kc in range(8):
                    nc.sync.dma_start(
                        out=xT[:, kc, b * N:(b + 1) * N],
                        in_=xT_ap[:, kc, b * N:(b + 1) * N])
            nc.sync.dma_start(out=wp[:], in_=wp_ap)

            nc.vector.memset(v_sb[:, :, 64:65], 1.0)
            nc.vector.memset(v_sb[:, :, 129:130], 1.0)

            def qkv_block(b):
                for tc_i in (2 * b, 2 * b + 1):
                    qps = ps_qk.tile([128, 512], F32, tag="qk",
                                     name=f"qps_{b}_{tc_i}")
                    kps = ps_qk.tile([128, 512], F32, tag="qk",
                                     name=f"kps_{b}_{tc_i}")
                    for kc in range(8):
                        _T("mm_qk", nc.tensor.matmul(qps[:], wqk[:, kc, 0:128],
                                         xT[:, kc, tc_i * 512:(tc_i + 1) * 512],
                                         start=(kc == 0), stop=(kc == 7)))
                        _T("mm_qk", nc.tensor.matmul(kps[:], wqk[:, kc, 128:256],
                                         xT[:, kc, tc_i * 512:(tc_i + 1) * 512],
                                         start=(kc == 0), stop=(kc == 7)))
                    nc.vector.tensor_copy(
                        out=q_sb[:, tc_i * 512:(tc_i + 1) * 512], in_=qps[:])
                    nc.vector.tensor_copy(
                        out=k_sb[:, tc_i * 512:(tc_i + 1) * 512], in_=kps[:])
                for tt in range(8 * b, 8 * b + 8):
                    vps = ps_qk.tile([128, 128], F32, tag="qk", name=f"vps_{tt}")
                    for kc in range(8):
                        _T("mm_v", nc.tensor.matmul(vps[:],
                                         xT[:, kc, tt * 128:(tt + 1) * 128],
                                         wv[:, kc, :],
                                         start=(kc == 0), stop=(kc == 7)))
                    nc.vector.tensor_copy(out=v_sb[:, tt, 0:64],
                                          in_=vps[:, 0:64])
                    nc.vector.tensor_copy(out=v_sb[:, tt, 65:129],
                                          in_=vps[:, 64:128])

            def attn_block(b, qt):
                q_sl = slice(b * N + qt * 512, b * N + (qt + 1) * 512)
                av0 = ps_av.tile([65, 512], F32, tag="av", name=f"av0_{b}_{qt}")
                av1 = ps_av.tile([65, 512], F32, tag="av", name=f"av1_{b}_{qt}")
                avs = [av0, av1]
                for kc in range(8):
                    k_sl = slice(b * N + kc * 128, b * N + (kc + 1) * 128)
                    sT = ps_sT.tile([128, 1024], F32, tag="sT",
                                    name=f"sT_{b}_{qt}_{kc}")
                    for hl in range(2):
                        _T("mm_sT", nc.tensor.matmul(
                            sT[:, hl * 512:(hl + 1) * 512],
                            k_sb[hl * 64:(hl + 1) * 64, k_sl],
                            q_sb[hl * 64:(hl + 1) * 64, q_sl],
                            start=True, stop=True,
                            tile_position=(hl * 64, 0)))
                    e = epool.tile([128, 1024], BF, tag="e",
                                   name=f"e_{b}_{qt}_{kc}")
                    _T("exp", nc.scalar.activation(
                        e[:], sT[:], mybir.ActivationFunctionType.Exp))
                    for hl in range(2):
                        _T("mm_av", nc.tensor.matmul(
                            avs[hl][:],
                            v_sb[:, 8 * b + kc, hl * 65:(hl + 1) * 65],
                            e[:, hl * 512:(hl + 1) * 512],
                            start=(kc == 0), stop=(kc == 7)))
                return avs

            def norm_block(b, qt, avs):
                # evacuate av psum -> sbuf (frees psum; DMA can then read den)
                av_sb = []
                for hl in range(2):
                    t = spool.tile([65, 512], F32, tag="avsb",
                                   name=f"avsb_{b}_{qt}_{hl}")
                    nc.vector.tensor_copy(out=t[:], in_=avs[hl][:])
                    av_sb.append(t)
                # one wide reciprocal for both heads: dens reshaped to [16,64]
                den = spool.tile([16, 64], F32, tag="den", name=f"den_{b}_{qt}")
                for hl in range(2):
                    nc.gpsimd.dma_start(out=den[8 * hl:8 * hl + 8, :],
                                        in_=av_sb[hl][64:65, :])
                rcp = spool.tile([16, 64], F32, tag="rcp", name=f"rcp_{b}_{qt}")
                nc.vector.reciprocal(rcp[:], den[:])
                db = dpool.tile([2, 512], F32, name=f"db_{b}_{qt}")
                nc.gpsimd.dma_start(out=db[:], in_=rcp[:])
                db_ap = db[:]
                for hl in range(2):
                    rb = spool.tile([64, 512], F32, tag="rbc",
                                    name=f"rb_{b}_{qt}_{hl}")
                    nc.gpsimd.dma_start(
                        out=rb[:],
                        in_=bass.AP(tensor=db_ap.tensor,
                                    offset=db_ap.offset + hl * 512,
                                    ap=[[0, 64], [1, 512]]))
                    nc.vector.tensor_mul(
                        projin[hl * 64:(hl + 1) * 64, b,
                               qt * 512:(qt + 1) * 512],
                        av_sb[hl][0:64, :], rb[:])

            def proj_wave(nts):
                for nt in nts:
                    pps0 = ps_qk.tile([128, 512], F32, tag="qk",
                                      name=f"pps0_{nt}")
                    pps1 = ps_qk.tile([128, 512], F32, tag="qk",
                                      name=f"pps1_{nt}")
                    for b2 in range(B):
                        _T("mm_proj", nc.tensor.matmul(
                            pps0[:], projin[:, b2, nt * 128:(nt + 1) * 128],
                            wp[:, b2, 0:512], start=(b2 == 0), stop=(b2 == 3)))
                        _T("mm_proj", nc.tensor.matmul(
                            pps1[:], projin[:, b2, nt * 128:(nt + 1) * 128],
                            wp[:, b2, 512:1024], start=(b2 == 0),
                            stop=(b2 == 3)))
                    for dt, pps in ((0, pps0), (1, pps1)):
                        ocp = opool.tile([128, 512], F32, tag="o",
                                         name=f"ocp_{nt}_{dt}")
                        if dt == 0:
                            nc.vector.tensor_copy(out=ocp[:], in_=pps[:])
                        else:
                            nc.scalar.activation(
                                ocp[:], pps[:],
                                mybir.ActivationFunctionType.Copy)
                        nc.sync.dma_start(
                            out=out_e[nt * 128:(nt + 1) * 128,
                                      dt * 512:(dt + 1) * 512],
                            in_=ocp[:])

            # schedule: qkv one batch ahead of attention to keep PE dense;
            # first half of proj (n<512 needs only qt=0 outputs) overlaps the
            # last attention block
            qkv_block(0)
            for b in range(B):
                if b + 1 < B:
                    qkv_block(b + 1)
                for qt in range(2):
                    avs = attn_block(b, qt)
                    norm_block(b, qt, avs)
                    if b == B - 1 and qt == 0:
                        proj_wave(range(0, 4))
            proj_wave(range(4, 8))

    nc.compile()
    return nc


def _prep_core(i, xT, w_qkv, w_proj):
    """Per-core input shards (host-side layout absorption)."""
    h0 = 2 * i
    rows = np.concatenate([np.arange(h0 * HD, (h0 + 1) * HD),
                           np.arange((h0 + 1) * HD, (h0 + 2) * HD)])
    w_qk = np.concatenate([w_qkv[rows] * 0.125, w_qkv[C + rows]], axis=0).T
    w_v = w_qkv[2 * C + rows].T
    hh = np.array([h0, h0 + 1])
    cg = ((hh % 4)[None, :, None] * 256
          + np.arange(B)[:, None, None] * 64
          + np.arange(HD)[None, None, :])          # [b2, hl, d]
    w_p = w_proj[:, cg.reshape(-1)].T              # [512, 1024]
    return {
        "xT": xT,
        "w_qk": np.ascontiguousarray(w_qk, dtype=bf16),
        "w_v": np.ascontiguousarray(w_v, dtype=bf16),
        "w_p": np.ascontiguousarray(w_p, dtype=bf16),
    }


def _run(inputs, trace=False, **kw):
    x = np.asarray(inputs["x"], dtype=np.float32)
    w_qkv = np.asarray(inputs["w_qkv"], dtype=np.float32)
    w_proj = np.asarray(inputs["w_proj"], dtype=np.float32)
    b_proj = np.asarray(inputs["b_proj"], dtype=np.float32)

    if "nc" not in _NC_CACHE:
        _NC_CACHE["nc"] = build_nc()
    nc = _NC_CACHE["nc"]

    xT = np.ascontiguousarray(
        x.transpose(2, 1, 0).reshape(C, NT), dtype=bf16)
    in_maps = [_prep_core(i, xT, w_qkv, w_proj) for i in range(NCORES)]
    res = run_bass_kernel_spmd(nc, in_maps, core_ids=list(range(NCORES)),
                               trace=trace, **kw)
    out = np.empty((N, B, C), np.float32)
    for j in range(4):
        out[:, j, :] = (res.results[2 * j]["out"]
                        + res.results[2 * j + 1]["out"] + b_proj)
    return out, res


def kernel(**inputs) -> np.ndarray:
    out, _ = _run(inputs, trace=False)
    return out


# revision 16
# speedup vs baseline: 1.0628x; 1.0628x over previous
"""Trainium2 8-core kernel for nn_Attention_88948772700322.

Reference computes (N=1024, B=4, C=1024, H=16, hd=64):
    qkv = x @ w_qkv.T                      [N,B,3C]
    q,k,v per (b,h); attn = softmax(q k^T / 8) v
    out = (attn.transpose(2,1,0,3)).reshape(N,B,C) @ w_proj.T + b_proj
The reshape interleaves H and B: proj-input channel c of output-batch bn is
attention head h = 4*bn + c//256, original batch b2 = (c%256)//64, dim d = c%64.

Sharding: tensor-parallel over heads — core i owns heads {2i, 2i+1}, all
batches/tokens (6.44 GFLOP/core, perfectly balanced).  Each core computes a
partial projection over its 512 proj-input channels for output batch bn=i//2;
host sums core pairs (the "all-reduce after proj" realized in unshard).

Host-side prep absorbs every layout nuisance:
  - xT [C, B*N] bf16, tokens batch-major  -> qkv needs no on-chip transpose
  - w_qk [C, 256] (cols q_h0,q_h1,k_h0,k_h1), q pre-scaled by 1/8
  - w_v  [C, 128] (cols v_h0,v_h1)
  - w_p  [512, 1024] = w_proj columns permuted to (b2, h_local, d) row order
On-chip per core: qk^T via PE (d-major), v via PE (token-major), scores
computed transposed (keys on partitions), softmax without max-subtraction
(scores are O(1) by construction), denominator via ones-column in V,
normalization by DMA-partition-broadcast reciprocal, partial proj n-major.
"""

import numpy as np
import ml_dtypes

import concourse.bass as bass
import concourse.mybir as mybir
from concourse import bacc
from concourse.tile import TileContext
from concourse.bass_utils import run_bass_kernel_spmd


N, B, C, H, HD = 1024, 4, 1024, 16, 64
NT = B * N          # 4096 tokens
NCORES = 8
BF = mybir.dt.bfloat16
F32 = mybir.dt.float32
bf16 = ml_dtypes.bfloat16

_NC_CACHE = {}
INST_PHASE = {}


def _T(phase, binst):
    INST_PHASE[binst.ins.name] = phase
    return binst


def _part_bcast(ap, nparts):
    """AP reading one partition, broadcast (step 0) across nparts partitions."""
    return bass.AP(tensor=ap.tensor, offset=ap.offset,
                   ap=[[0, nparts]] + list(ap.ap)[1:])


def build_nc():
    nc = bacc.Bacc()
    xT_e = nc.declare_dram_parameter("xT", [C, NT], BF, isOutput=False)
    wqk_e = nc.declare_dram_parameter("w_qk", [C, 256], BF, isOutput=False)
    wv_e = nc.declare_dram_parameter("w_v", [C, 128], BF, isOutput=False)
    wp_e = nc.declare_dram_parameter("w_p", [512, C], BF, isOutput=False)
    out_e = nc.declare_dram_parameter("out", [N, C], F32, isOutput=True)

    xT_ap = xT_e[:].rearrange("(co p) t -> p co t", p=128)    # [128, 8, 4096]
    wqk_ap = wqk_e[:].rearrange("(co p) m -> p co m", p=128)  # [128, 8, 256]
    wv_ap = wv_e[:].rearrange("(co p) m -> p co m", p=128)    # [128, 8, 128]
    wp_ap = wp_e[:].rearrange("(b2 p) d -> p b2 d", p=128)    # [128, 4, 1024]

    from contextlib import ExitStack
    with TileContext(nc) as tc:
        with ExitStack() as stk:
            cpool = stk.enter_context(tc.tile_pool(name="const", bufs=1))
            epool = stk.enter_context(tc.tile_pool(name="exp", bufs=4))
            spool = stk.enter_context(tc.tile_pool(name="small", bufs=4))
            opool = stk.enter_context(tc.tile_pool(name="outcp", bufs=4))
            dpool = stk.enter_context(
                tc.tile_pool(name="dram", bufs=2, space="DRAM"))
            attn_stk = ExitStack()
            ps_qk = attn_stk.enter_context(
                tc.tile_pool(name="ps_qk", bufs=2, space="PSUM"))
            ps_sT = attn_stk.enter_context(
                tc.tile_pool(name="ps_sT", bufs=2, space="PSUM"))
            ps_av = attn_stk.enter_context(
                tc.tile_pool(name="ps_av", bufs=2, space="PSUM"))
            # ---- persistent SBUF tensors -------------------------------
            xT = cpool.tile([128, 8, NT], BF)      # x^T  [c_in, co, token]
            wqk = cpool.tile([128, 8, 256], BF)
            wv = cpool.tile([128, 8, 128], BF)
            wp = cpool.tile([128, 4, C], BF)
            q_sb = cpool.tile([128, NT], BF)       # [ (h0|h1) d, token ]
            k_sb = cpool.tile([128, NT], BF)
            # v token-major with ones cols: [t_in, t_out, (1,h0 d64,1,h1 d64)]
            v_sb = cpool.tile([128, 32, 130], BF)
            projin = cpool.tile([128, B, N], BF)   # [(hl,d), b2, n]

            nc.sync.dma_start(out=wqk[:], in_=wqk_ap)
            nc.sync.dma_start(out=wv[:], in_=wv_ap)
            # x chunks in batch-major order so batch 0 compute starts early
            for b in range(B):
                for kc in range(8):
                    nc.sync.dma_start(
                        out=xT[:, kc, b * N:(b + 1) * N],
                        in_=xT_ap[:, kc, b * N:(b + 1) * N])
            nc.sync.dma_start(out=wp[:], in_=wp_ap)

            nc.vector.memset(v_sb[:, :, 64:65], 1.0)
            nc.vector.memset(v_sb[:, :, 129:130], 1.0)

            def qkv_block(b):
                for tc_i in (2 * b, 2 * b + 1):
                    qps = ps_qk.tile([128, 512], F32, tag="qk",
                                     name=f"qps_{b}_{tc_i}")
                    kps = ps_qk.tile([128, 512], F32, tag="qk",
                                     name=f"kps_{b}_{tc_i}")
                    for kc in range(8):
                        _T("mm_qk", nc.tensor.matmul(qps[:], wqk[:, kc, 0:128],
                                         xT[:, kc, tc_i * 512:(tc_i + 1) * 512],
                                         start=(kc == 0), stop=(kc == 7)))
                        _T("mm_qk", nc.tensor.matmul(kps[:], wqk[:, kc, 128:256],
                                         xT[:, kc, tc_i * 512:(tc_i + 1) * 512],
                                         start=(kc == 0), stop=(kc == 7)))
                    nc.vector.tensor_copy(
                        out=q_sb[:, tc_i * 512:(tc_i + 1) * 512], in_=qps[:])
                    nc.vector.tensor_copy(
                        out=k_sb[:, tc_i * 512:(tc_i + 1) * 512], in_=kps[:])
                for tt in range(8 * b, 8 * b + 8):
                    vps = ps_qk.tile([128, 128], F32, tag="qk", name=f"vps_{tt}")
                    for kc in range(8):
                        _T("mm_v", nc.tensor.matmul(vps[:],
                                         xT[:, kc, tt * 128:(tt + 1) * 128],
                                         wv[:, kc, :],
                                         start=(kc == 0), stop=(kc == 7)))
                    nc.vector.tensor_copy(out=v_sb[:, tt, 0:64],
                                          in_=vps[:, 0:64])
                    nc.vector.tensor_copy(out=v_sb[:, tt, 65:129],
                                          in_=vps[:, 64:128])

            def attn_block(b, qt):
                q_sl = slice(b * N + qt * 512, b * N + (qt + 1) * 512)
                av0 = ps_av.tile([65, 512], F32, tag="av", name=f"av0_{b}_{qt}")
                av1 = ps_av.tile([65, 512], F32, tag="av", name=f"av1_{b}_{qt}")
                avs = [av0, av1]
                for kc in range(8):
                    k_sl = slice(b * N + kc * 128, b * N + (kc + 1) * 128)
                    sT = ps_sT.tile([128, 1024], F32, tag="sT",
                                    name=f"sT_{b}_{qt}_{kc}")
                    for hl in range(2):
                        _T("mm_sT", nc.tensor.matmul(
                            sT[:, hl * 512:(hl + 1) * 512],
                            k_sb[hl * 64:(hl + 1) * 64, k_sl],
                            q_sb[hl * 64:(hl + 1) * 64, q_sl],
                            start=True, stop=True,
                            tile_position=(hl * 64, 0)))
                    e = epool.tile([128, 1024], BF, tag="e",
                                   name=f"e_{b}_{qt}_{kc}")
                    _T("exp", nc.scalar.activation(
                        e[:], sT[:], mybir.ActivationFunctionType.Exp))
                    for hl in range(2):
                        _T("mm_av", nc.tensor.matmul(
                            avs[hl][:],
                            v_sb[:, 8 * b + kc, hl * 65:(hl + 1) * 65],
                            e[:, hl * 512:(hl + 1) * 512],
                            start=(kc == 0), stop=(kc == 7)))
                return avs

            def norm_block(b, qt, avs):
                # evacuate av psum -> sbuf (frees psum; DMA can then read den)
                av_sb = []
                for hl in range(2):
                    t = spool.tile([65, 512], F32, tag="avsb",
                                   name=f"avsb_{b}_{qt}_{hl}")
                    nc.vector.tensor_copy(out=t[:], in_=avs[hl][:])
                    av_sb.append(t)
                # one wide reciprocal for both heads: dens reshaped to [16,64]
                den = spool.tile([16, 64], F32, tag="den", name=f"den_{b}_{qt}")
                for hl in range(2):
                    nc.gpsimd.dma_start(out=den[8 * hl:8 * hl + 8, :],
                                        in_=av_sb[hl][64:65, :])
                rcp = spool.tile([16, 64], F32, tag="rcp", name=f"rcp_{b}_{qt}")
                nc.vector.reciprocal(rcp[:], den[:])
                db = dpool.tile([2, 512], F32, name=f"db_{b}_{qt}")
                nc.gpsimd.dma_start(out=db[:], in_=rcp[:])
                db_ap = db[:]
                for hl in range(2):
                    rb = spool.tile([64, 512], F32, tag="rbc",
                                    name=f"rb_{b}_{qt}_{hl}")
                    nc.gpsimd.dma_start(
                        out=rb[:],
                        in_=bass.AP(tensor=db_ap.tensor,
                                    offset=db_ap.offset + hl * 512,
                                    ap=[[0, 64], [1, 512]]))
                    nc.vector.tensor_mul(
                        projin[hl * 64:(hl + 1) * 64, b,
                               qt * 512:(qt + 1) * 512],
                        av_sb[hl][0:64, :], rb[:])

            def proj_wave(nts, pool, ptag):
                for nt in nts:
                    pps0 = pool.tile([128, 512], F32, tag=ptag,
                                     name=f"pps0_{nt}")
                    pps1 = pool.tile([128, 512], F32, tag=ptag,
                                     name=f"pps1_{nt}")
                    for b2 in range(B):
                        _T("mm_proj", nc.tensor.matmul(
                            pps0[:], projin[:, b2, nt * 128:(nt + 1) * 128],
                            wp[:, b2, 0:512], start=(b2 == 0), stop=(b2 == 3)))
                        _T("mm_proj", nc.tensor.matmul(
                            pps1[:], projin[:, b2, nt * 128:(nt + 1) * 128],
                            wp[:, b2, 512:1024], start=(b2 == 0),
                            stop=(b2 == 3)))
                    for dt, pps in ((0, pps0), (1, pps1)):
                        ocp = opool.tile([128, 512], F32, tag="o",
                                         name=f"ocp_{nt}_{dt}")
                        nc.vector.tensor_copy(out=ocp[:], in_=pps[:])
                        nc.sync.dma_start(
                            out=out_e[nt * 128:(nt + 1) * 128,
                                      dt * 512:(dt + 1) * 512],
                            in_=ocp[:])

            # schedule: qkv one batch ahead of attention to keep PE dense;
            # first half of proj (n<512 needs only qt=0 outputs) overlaps the
            # last attention block
            qkv_block(0)
            for b in range(B):
                if b + 1 < B:
                    qkv_block(b + 1)
                for qt in range(2):
                    avs = attn_block(b, qt)
                    norm_block(b, qt, avs)
                    if b == B - 1 and qt == 0:
                        proj_wave(range(0, 4), ps_qk, "qk")
            attn_stk.close()
            with tc.tile_pool(name="ps_proj", bufs=6, space="PSUM") as ps_proj:
                proj_wave(range(4, 8), ps_proj, "pp")

    nc.compile()
    return nc


def _prep_core(i, xT, w_qkv, w_proj):
    """Per-core input shards (host-side layout absorption)."""
    h0 = 2 * i
    rows = np.concatenate([np.arange(h0 * HD, (h0 + 1) * HD),
                           np.arange((h0 + 1) * HD, (h0 + 2) * HD)])
    w_qk = np.concatenate([w_qkv[rows] * 0.125, w_qkv[C + rows]], axis=0).T
    w_v = w_qkv[2 * C + rows].T
    hh = np.array([h0, h0 + 1])
    cg = ((hh % 4)[None, :, None] * 256
          + np.arange(B)[:, None, None] * 64
          + np.arange(HD)[None, None, :])          # [b2, hl, d]
    w_p = w_proj[:, cg.reshape(-1)].T              # [512, 1024]
    return {
        "xT": xT,
        "w_qk": np.ascontiguousarray(w_qk, dtype=bf16),
        "w_v": np.ascontiguousarray(w_v, dtype=bf16),
        "w_p": np.ascontiguousarray(w_p, dtype=bf16),
    }


def _run(inputs, trace=False, **kw):
    x = np.asarray(inputs["x"], dtype=np.float32)
    w_qkv = np.asarray(inputs["w_qkv"], dtype=np.float32)
    w_proj = np.asarray(inputs["w_proj"], dtype=np.float32)
    b_proj = np.asarray(inputs["b_proj"], dtype=np.float32)

    if "nc" not in _NC_CACHE:
        _NC_CACHE["nc"] = build_nc()
    nc = _NC_CACHE["nc"]

    xT = np.ascontiguousarray(
        x.transpose(2, 1, 0).reshape(C, NT), dtype=bf16)
    in_maps = [_prep_core(i, xT, w_qkv, w_proj) for i in range(NCORES)]
    res = run_bass_kernel_spmd(nc, in_maps, core_ids=list(range(NCORES)),
                               trace=trace, **kw)
    out = np.empty((N, B, C), np.float32)
    for j in range(4):
        out[:, j, :] = (res.results[2 * j]["out"]
                        + res.results[2 * j + 1]["out"] + b_proj)
    return out, res


def kernel(**inputs) -> np.ndarray:
    out, _ = _run(inputs, trace=False)
    return out
